# revision 1
# baseline (speedup 1.0000x reference)
"""Fused QKV + RMSNorm + RoPE + self-attention kernel for Trainium2.

Sharding: tensor-parallel over heads. 16 heads / 8 cores = 2 heads per core.
Each core computes qkv projection for its 2 heads (column-parallel on the
3*dim output), per-head RMSNorm/RoPE/attention locally, and writes its
[B, N, 256] slice of the output. The host concatenates slices (the output
projection is absent, so the "all-gather" is a host-side concat).

Host-side weight preprocessing:
  - the reference layout interleaves q/k/v at stride 3 per (head, dim):
    row = h*384 + d*3 + j.  We de-interleave by permuting w_qkv rows.
  - q/k head-dims are permuted even-first ([0,2,..,126,1,3,..,127]) so the
    interleaved RoPE becomes rotate-half style.  Scores q.k are invariant
    under a common permutation of q and k head-dims (RMSNorm too), and v is
    left unpermuted, so the final output is unchanged.

Compute dtype: bf16 matmuls with fp32 accumulation; norm/softmax math fp32.
"""

import sys

sys.path.insert(0, "/opt/trn_rl_repo")

import numpy as np
import ml_dtypes

import concourse.bass as bass
import concourse.mybir as mybir
import concourse.tile as tile
from concourse import bacc
from concourse.masks import make_identity

B = 2
SEQ = 2048
DIM = 2048
NHEADS = 16
HEAD_DIM = 128
NCORES = 8
HPC = NHEADS // NCORES  # heads per core = 2
EPS = 1e-6
SCALE = float(HEAD_DIM) ** -0.5
P = 128  # partitions

F32 = mybir.dt.float32
BF16 = mybir.dt.bfloat16
F32R = mybir.dt.float32r
I32 = mybir.dt.int32

QG = 512  # q tokens per attention inner group


def build_nc(seq=SEQ, batches=B):
    """Build the SPMD per-core graph. Same graph runs on all 8 cores."""
    tokens = batches * seq
    nt = tokens // P  # token tiles overall
    ntb = seq // P  # token tiles per batch
    kc_n = DIM // P  # contraction chunks for qkv projection (16)
    fpc = 3 * HPC * HEAD_DIM  # per-core projection output features = 768
    qg_per = seq // QG  # q groups per (b, h)
    gq = QG // P  # 128-tiles per q group (4)

    nc = bacc.Bacc(None, target_bir_lowering=False)

    xt_ext = nc.declare_dram_parameter("xt", [DIM, tokens], BF16, isOutput=False)
    wt_ext = nc.declare_dram_parameter("wt", [DIM, fpc], BF16, isOutput=False)
    bias_ext = nc.declare_dram_parameter("bias", [1, fpc], F32, isOutput=False)
    cs_ext = nc.declare_dram_parameter("cs", [seq, 128], BF16, isOutput=False)
    sc_ext = nc.declare_dram_parameter("sc", [seq, 128], BF16, isOutput=False)
    out_ext = nc.declare_dram_parameter(
        "out", [batches, seq, HPC * HEAD_DIM], F32, isOutput=True
    )

    add = mybir.AluOpType.add
    sub = mybir.AluOpType.subtract
    mul = mybir.AluOpType.mult

    with tile.TileContext(nc) as tc:
        with (
            tc.tile_pool(name="consts", bufs=1) as consts,
            tc.tile_pool(name="persist", bufs=1) as persist,
        ):
            ident = consts.tile([P, P], BF16, tag="ident")
            make_identity(nc, ident[:])
            ones_col = consts.tile([P, 1], BF16, tag="ones")
            nc.vector.memset(ones_col[:], 1.0)
            eps_sb = consts.tile([P, 1], F32, tag="eps")
            nc.vector.memset(eps_sb[:], EPS)

            wt_sb = consts.tile([P, kc_n, fpc], BF16, tag="wt")
            wt_r = wt_ext[:].rearrange("(kc p) f -> p kc f", p=P)
            for kc in range(4):
                nc.sync.dma_start(out=wt_sb[:, kc, :], in_=wt_r[:, kc, :])

            bias_sb = consts.tile([P, fpc], F32, tag="bias")
            cs_sb = consts.tile([P, ntb, P], BF16, tag="cs")
            sc_sb = consts.tile([P, ntb, P], BF16, tag="sc")

            bap = bias_ext[:]
            bias_bcast = bass.AP(
                tensor=bap.tensor, offset=bap.offset, ap=[[0, P], [1, fpc]]
            )
            nc.sync.dma_start(out=bias_sb[:], in_=bias_bcast)
            cs_r = cs_ext[:].rearrange("(ti p) d -> p ti d", p=P)
            sc_r = sc_ext[:].rearrange("(ti p) d -> p ti d", p=P)

            # persistent per-(batch, local-head) attention operands
            # qT/kT feature-major: [d, tile, tok]; v token-major: [tok, chunk, d]
            qT = {}
            kT = {}
            vv = {}
            for b in range(batches):
                for hl in range(HPC):
                    qT[(b, hl)] = persist.tile([P, ntb, P], BF16, tag=f"qT{b}_{hl}", name=f"qT{b}_{hl}")
                    kT[(b, hl)] = persist.tile([P, ntb, P], BF16, tag=f"kT{b}_{hl}", name=f"kT{b}_{hl}")
                    vv[(b, hl)] = persist.tile([P, ntb, P], BF16, tag=f"v{b}_{hl}", name=f"v{b}_{hl}")

            # Phase 1 (projection+norm+rope) and phase 2 (attention) share
            # pools and are emitted interleaved per batch, so batch b+1's
            # PE-heavy projection overlaps batch b's ACT-heavy softmax.
            with (
                tc.tile_pool(name="p1", bufs=2) as p1,
                tc.tile_pool(name="p1s", bufs=2) as p1s,
                tc.tile_pool(name="p2", bufs=2) as p2,
                tc.tile_pool(name="dramp", bufs=2, space="DRAM") as dramp,
                tc.tile_pool(name="psp", bufs=1, space="PSUM") as psp,
            ):
                xt_r = xt_ext[:].rearrange("(kc p) n -> p kc n", p=P)

                def phase1_tile(b_idx, ti):
                    t = b_idx * ntb + ti
                    x_tile = p1.tile([P, kc_n, P], BF16, tag="x", name="x_tile")
                    qc = kc_n // 4
                    for xq in range(4):
                        nc.sync.dma_start(
                            out=x_tile[:, xq * qc : (xq + 1) * qc, :],
                            in_=xt_r[:, xq * qc : (xq + 1) * qc, t * P : (t + 1) * P],
                        )
                    if b_idx == 0 and ti == 0:
                        for kc in range(4, kc_n):
                            nc.sync.dma_start(
                                out=wt_sb[:, kc, :], in_=wt_r[:, kc, :]
                            )
                    if b_idx == 0:
                        nc.sync.dma_start(out=cs_sb[:, ti, :], in_=cs_r[:, ti, :])
                        nc.sync.dma_start(out=sc_sb[:, ti, :], in_=sc_r[:, ti, :])
                    ps_a = psp.tile([P, 512], F32, tag="psA", bufs=2, name="ps_a")
                    ps_b = psp.tile([P, 256], F32, tag="psB", bufs=1, name="ps_b")
                    for kc in range(kc_n):
                        st = kc == 0
                        sp = kc == kc_n - 1
                        nc.tensor.matmul(
                            ps_a[:],
                            x_tile[:, kc, :],
                            wt_sb[:, kc, 0:512],
                            start=st,
                            stop=sp,
                        )
                        nc.tensor.matmul(
                            ps_b[:],
                            x_tile[:, kc, :],
                            wt_sb[:, kc, 512:768],
                            start=st,
                            stop=sp,
                        )
                    qkv_sb = p1.tile([P, fpc], F32, tag="qkv")
                    nc.vector.tensor_tensor(
                        qkv_sb[:, 0:512], ps_a[:], bias_sb[:, 0:512], add
                    )
                    nc.vector.tensor_tensor(
                        qkv_sb[:, 512:768], ps_b[:], bias_sb[:, 512:768], add
                    )

                    cs = cs_sb[:, ti, :]
                    sn = sc_sb[:, ti, :]
                    # rms stats for the 4 q/k blocks, then one batched
                    # sqrt+reciprocal for the tile
                    ms = p1s.tile([P, 4], F32, tag="ms")
                    for blk in range(4):
                        c0 = blk * P
                        xb = qkv_sb[:, c0 : c0 + P]
                        sq = p1s.tile([P, P], F32, tag="sq")
                        if b_idx == 0:
                            nc.scalar.activation(
                                out=sq[:],
                                in_=xb,
                                func=mybir.ActivationFunctionType.Square,
                                accum_out=ms[:, blk : blk + 1],
                            )
                        else:
                            nc.vector.scalar_tensor_tensor(
                                sq[:], xb, 1.0, xb, mul, mul,
                                accum_out=ms[:, blk : blk + 1],
                            )
                    # rstd = 1/sqrt(ms/128 + eps) via bit-trick + one
                    # Newton step, all on DVE (keeps ACT exp-only: no
                    # activation-table thrashing)
                    aa = p1s.tile([P, 4], F32, tag="aa")
                    nc.vector.tensor_scalar(
                        aa[:], ms[:], 1.0 / HEAD_DIM, EPS, mul, add
                    )
                    y0i = p1s.tile([P, 4], I32, tag="y0i")
                    nc.vector.tensor_scalar(
                        y0i[:], aa[:].bitcast(I32), 1, None,
                        mybir.AluOpType.logical_shift_right,
                    )
                    nc.vector.tensor_scalar(
                        y0i[:], y0i[:], -1, 0x5F3759DF, mul, add
                    )
                    y0 = y0i[:].bitcast(F32)
                    t1 = p1s.tile([P, 4], F32, tag="t1")
                    nc.vector.tensor_tensor(t1[:], y0, y0, mul)
                    nc.vector.scalar_tensor_tensor(
                        t1[:], t1[:], -0.5, aa[:], mul, mul
                    )
                    rstd = p1s.tile([P, 4], F32, tag="rstd")
                    nc.vector.scalar_tensor_tensor(
                        rstd[:], t1[:], 1.5, y0, add, mul
                    )
                    # second Newton step for accuracy
                    nc.vector.tensor_tensor(t1[:], rstd[:], rstd[:], mul)
                    nc.vector.scalar_tensor_tensor(
                        t1[:], t1[:], -0.5, aa[:], mul, mul
                    )
                    nc.vector.scalar_tensor_tensor(
                        rstd[:], t1[:], 1.5, rstd[:], add, mul
                    )
                    # blocks: 0 q_h0, 1 q_h1, 2 k_h0, 3 k_h1 (cols blk*128)
                    for blk in range(4):
                        c0 = blk * P
                        xb = qkv_sb[:, c0 : c0 + P]
                        # fused norm+rope: m12 = [(xb*rstd)*[c|s] |
                        # (xb*rstd)*[-s|c]]; roped = [m1_lo - m1_hi |
                        # m2_hi - m2_lo] via one strided subtract
                        m12 = p1s.tile([P, 2 * P], F32, tag="m12")
                        roped = p1s.tile([P, P], BF16, tag="roped")
                        nc.vector.scalar_tensor_tensor(
                            m12[:, 0:P], xb, rstd[:, blk : blk + 1], cs, mul, mul
                        )
                        nc.vector.scalar_tensor_tensor(
                            m12[:, P : 2 * P], xb, rstd[:, blk : blk + 1], sn,
                            mul, mul,
                        )
                        mb = m12[:]
                        a_ap = bass.AP(
                            tensor=mb.tensor, offset=mb.offset,
                            ap=[list(mb.ap[0]), [192, 2], [1, 64]],
                        )
                        b_ap = bass.AP(
                            tensor=mb.tensor, offset=mb.offset + 64,
                            ap=[list(mb.ap[0]), [64, 2], [1, 64]],
                        )
                        nc.vector.tensor_tensor(
                            roped[:].rearrange("p (a c) -> p a c", a=2),
                            a_ap, b_ap, sub,
                        )
                        # transpose to feature-major and store
                        tp = psp.tile([P, P], BF16, tag="small", bufs=1, name="tp")
                        nc.tensor.transpose(tp[:], roped[:], ident[:])
                        dest = qT if blk < 2 else kT
                        hl = blk % 2
                        nc.vector.tensor_copy(
                            dest[(b_idx, hl)][:, ti, :], tp[:]
                        )
                    for hl in range(HPC):
                        c0 = 512 + hl * P
                        nc.gpsimd.tensor_copy(
                            vv[(b_idx, hl)][:, ti, :], qkv_sb[:, c0 : c0 + P]
                        )

                def phase2_qgroup(b, hl, qg):
                    q_t = qT[(b, hl)]
                    k_t = kT[(b, hl)]
                    v_t = vv[(b, hl)]
                    qs_ap = q_t[:, qg * gq : (qg + 1) * gq, :]
                    probsT = p2.tile([P, ntb, QG], BF16, tag="probsT", bufs=3, name="probsT")
                    for kc in range(ntb):
                        s_ps = psp.tile([P, QG], F32, tag="sps", bufs=3, name="s_ps")
                        nc.tensor.matmul(
                            s_ps[:],
                            k_t[:, kc, :],
                            qs_ap,
                            start=True,
                            stop=True,
                        )
                        nc.scalar.activation(
                            out=probsT[:, kc, :],
                            in_=s_ps[:],
                            func=mybir.ActivationFunctionType.Exp,
                            scale=SCALE,
                        )
                    # AV: accumulate over k chunks
                    av_ps = psp.tile([P, QG], F32, tag="av", bufs=1, name="av_ps")
                    for kc in range(ntb):
                        nc.tensor.matmul(
                            av_ps[:],
                            v_t[:, kc, :],
                            probsT[:, kc, :],
                            start=(kc == 0),
                            stop=(kc == ntb - 1),
                        )
                    # sums over k: pairwise folds (wide ones in bf16, final
                    # ones in f32), then a ones-matmul per 128-q slice for the
                    # partition sum (f32r: full-rate fp32 path)
                    cur = probsT[:].rearrange("p a b -> p (a b)")
                    width = ntb * QG
                    lvl = 0
                    while width > QG:
                        width //= 2
                        dt_out = BF16
                        nxt = p2.tile([P, width], dt_out, tag=f"fold{lvl}", bufs=1, name="fold")
                        nc.vector.tensor_tensor(
                            nxt[:],
                            cur[:, 0:width],
                            cur[:, width : 2 * width],
                            add,
                        )
                        cur = nxt[:]
                        lvl += 1
                    sums = cur
                    scol = psp.tile([P, gq], F32, tag="small", bufs=1, name="scol")
                    for qs in range(gq):
                        nc.tensor.matmul(
                            scol[:, qs : qs + 1],
                            sums[:, qs * P : (qs + 1) * P],
                            ones_col[:],
                            start=True,
                            stop=True,
                            skip_group_check=True,
                        )
                    recip = p2.tile([P, gq], F32, tag="recip", name="recip")
                    nc.vector.reciprocal(recip[:], scol[:])
                    # evac AV, transpose to token-major, normalize
                    av_sb = p2.tile([P, QG], BF16, tag="avsb", name="av_sb")
                    nc.vector.tensor_copy(av_sb[:], av_ps[:])
                    out_sb = p2.tile([P, gq, P], F32, tag="outsb", name="out_sb")
                    for qs in range(gq):
                        otp = psp.tile([P, P], BF16, tag="small", bufs=1, name="otp")
                        nc.tensor.transpose(
                            otp[:], av_sb[:, qs * P : (qs + 1) * P], ident[:]
                        )
                        nc.vector.tensor_scalar_mul(
                            out_sb[:, qs, :], otp[:], recip[:, qs : qs + 1]
                        )
                    dest = out_ext[
                        b, qg * QG : (qg + 1) * QG, hl * P : (hl + 1) * P
                    ].rearrange("(qs p) d -> p qs d", p=P)
                    nc.sync.dma_start(out=dest, in_=out_sb[:])

                # interleaved emission: batch b's projection tiles are woven
                # between batch b-1's attention qgroups so PE-heavy and
                # ACT-heavy work stay concurrently available to the scheduler
                p2_units = {
                    b: [(b, hl, qg) for qg in range(qg_per) for hl in range(HPC)]
                    for b in range(batches)
                }
                for ti in range(ntb):
                    phase1_tile(0, ti)
                for b in range(1, batches):
                    prev = p2_units[b - 1]
                    ratio = max(1, ntb // max(1, len(prev)))
                    pi = 0
                    for ti in range(ntb):
                        phase1_tile(b, ti)
                        if (ti + 1) % ratio == 0 and pi < len(prev):
                            phase2_qgroup(*prev[pi])
                            pi += 1
                    while pi < len(prev):
                        phase2_qgroup(*prev[pi])
                        pi += 1
                for u in p2_units[batches - 1]:
                    phase2_qgroup(*u)

    nc.compile()
    return nc


def prep_inputs(x, w_qkv, b_qkv, cos, sin):
    """Build per-core input maps (host-side sharding)."""
    bf16 = ml_dtypes.bfloat16
    batches, seq, dim = x.shape
    xt = np.ascontiguousarray(
        x.reshape(batches * seq, dim).T.astype(bf16)
    )  # [DIM, tokens]
    cosf = cos.astype(np.float32)
    sinf = sin.astype(np.float32)
    csf = np.ascontiguousarray(np.concatenate([cosf, sinf], axis=1).astype(bf16))
    scf = np.ascontiguousarray(np.concatenate([-sinf, cosf], axis=1).astype(bf16))
    dperm = np.concatenate([np.arange(0, HEAD_DIM, 2), np.arange(1, HEAD_DIM, 2)])
    dnat = np.arange(HEAD_DIM)
    in_maps = []
    for c in range(NCORES):
        h0, h1 = HPC * c, HPC * c + 1
        idx = np.concatenate(
            [
                h0 * 384 + dperm * 3 + 0,
                h1 * 384 + dperm * 3 + 0,
                h0 * 384 + dperm * 3 + 1,
                h1 * 384 + dperm * 3 + 1,
                h0 * 384 + dnat * 3 + 2,
                h1 * 384 + dnat * 3 + 2,
            ]
        )
        wt = np.ascontiguousarray(w_qkv[idx, :].T.astype(bf16))  # [DIM, 768]
        bb = np.ascontiguousarray(b_qkv[idx].astype(np.float32)[None, :])
        in_maps.append(
            {"xt": xt, "wt": wt, "bias": bb, "cs": csf, "sc": scf}
        )
    return in_maps


_CACHED = {}


def _get_nc(seq, batches):
    key = (seq, batches)
    if key not in _CACHED:
        _CACHED[key] = build_nc(seq, batches)
    return _CACHED[key]


def run(x, w_qkv, b_qkv, cos, sin, trace=False):
    from concourse.bass_utils import run_bass_kernel_spmd

    batches, seq, _ = x.shape
    nc = _get_nc(seq, batches)
    in_maps = prep_inputs(x, w_qkv, b_qkv, cos, sin)
    res = run_bass_kernel_spmd(
        nc, in_maps, core_ids=list(range(NCORES)), trace=trace
    )
    out = np.concatenate([res.results[c]["out"] for c in range(NCORES)], axis=-1)
    return out.astype(np.float32), res


def kernel(x, w_qkv, b_qkv, cos, sin):
    out, _ = run(
        np.asarray(x),
        np.asarray(w_qkv),
        np.asarray(b_qkv),
        np.asarray(cos),
        np.asarray(sin),
        trace=False,
    )
    return out

